# revision 16
# baseline (speedup 1.0000x reference)
"""BinaryTreeLSTM on 8 Trainium2 NeuronCores.

Data-parallel over the leaf batch: core d owns leaves [1024d, 1024d+1024)
and folds its subtree through 10 merge levels; the 8 per-core roots are
AllGathered and the final 3 levels run replicated on every core.

Two matmul regimes (fp32r operands, single-pass PE):
- Feature-major (leaf, B=512, B=256 levels): weights stationary, nodes
  on the moving free dim. State h is kept as [128, 2 chunks * B] with
  even/odd children split into separate tiles so weight loads and reads
  stay contiguous.
- Node-major (B <= 128 levels): h chunks stationary (tiny weight loads),
  W streams as the moving operand in 512-wide chunks. Gates/c/h are
  node-major [B, 256]; h is transposed back to feature-major via PE
  transposes for the next level, and lc/rc come from partition-strided
  SBUF DMAs of the previous node-major c.
"""

import numpy as np

IN_DIM = 300
MEM_DIM = 256
N_LEAVES = 8192
N_CORES = 8
LPC = N_LEAVES // N_CORES  # 1024 leaves per core

# FM-gate m-chunk (5-gate [u,i,lf,rf,o] x 2 halves) -> column of the
# [128, 8] feature-major pad_xg ([cx,ix,fx,ox]; lf and rf share fx)
_PXCOL = [0, 1, 2, 3, 4, 5, 4, 5, 6, 7]
# node-major 5-gate px layout offsets into the 4-gate [1,1024] px row
_PX5SRC = [0, 256, 512, 512, 768]

_CACHE = {}


def _build():
    import concourse.bacc as bacc
    import concourse.mybir as mybir
    import concourse.tile as tile

    f32 = mybir.dt.float32
    f32r = mybir.dt.float32r
    AF = mybir.ActivationFunctionType

    nc = bacc.Bacc("TRN2", target_bir_lowering=False, debug=False,
                   num_devices=N_CORES)

    embsT = nc.dram_tensor("embsT", [IN_DIM, LPC], f32r, kind="ExternalInput").ap()
    WxT = nc.dram_tensor("WxT", [IN_DIM, 1024], f32r, kind="ExternalInput").ap()
    WlT = nc.dram_tensor("WlT", [MEM_DIM, 1280], f32r, kind="ExternalInput").ap()
    WrT = nc.dram_tensor("WrT", [MEM_DIM, 1280], f32r, kind="ExternalInput").ap()
    bxr = nc.dram_tensor("bxr", [1, 1024], f32, kind="ExternalInput").ap()
    padT = nc.dram_tensor("padT", [IN_DIM, 1], f32r, kind="ExternalInput").ap()
    eye_in = nc.dram_tensor("eye_in", [128, 128], f32, kind="ExternalInput").ap()
    ones_in = nc.dram_tensor("ones_in", [1, 128], f32r, kind="ExternalInput").ap()
    out = nc.dram_tensor("out", [2, 1, MEM_DIM], f32, kind="ExternalOutput").ap()

    with tile.TileContext(nc) as tc:
        with (
            tc.tile_pool(name="const", bufs=1) as const,
            tc.tile_pool(name="state", bufs=2) as state,
            tc.tile_pool(name="gates", bufs=2) as gates,
            tc.tile_pool(name="psum", bufs=2, space="PSUM") as psum,
            tc.tile_pool(name="dram", bufs=1, space="DRAM") as dram,
        ):
            v2 = lambda t: t.rearrange("p (c n) -> p c n", c=2)

            # ---- constants ----
            WxT_sb = const.tile([128, 3 * 1024], f32r)
            embsT_sb = const.tile([128, 3 * LPC], f32r)
            for k in range(3):
                r = 128 if k < 2 else IN_DIM - 256
                nc.sync.dma_start(WxT_sb[0:r, k * 1024:(k + 1) * 1024],
                                  WxT[128 * k:128 * k + r, :])
                nc.sync.dma_start(embsT_sb[0:r, k * LPC:(k + 1) * LPC],
                                  embsT[128 * k:128 * k + r, :])
            WlT_sb = const.tile([128, 2 * 1280], f32r)
            WrT_sb = const.tile([128, 2 * 1280], f32r)
            for k in range(2):
                nc.sync.dma_start(WlT_sb[:, k * 1280:(k + 1) * 1280],
                                  WlT[128 * k:128 * (k + 1), :])
                nc.sync.dma_start(WrT_sb[:, k * 1280:(k + 1) * 1280],
                                  WrT[128 * k:128 * (k + 1), :])
            bx_sb = const.tile([1, 1024], f32)
            nc.sync.dma_start(bx_sb[:, :], bxr[:, :])
            bx_fm = const.tile([128, 8], f32)
            nc.sync.dma_start(bx_fm[:, :],
                              bxr.rearrange("o (m p) -> p (o m)", p=128))
            padT_sb = const.tile([128, 3], f32r)
            for k in range(3):
                r = 128 if k < 2 else IN_DIM - 256
                nc.sync.dma_start(padT_sb[0:r, k:k + 1], padT[128 * k:128 * k + r, :])
            eye_sb = const.tile([128, 128], f32)
            nc.sync.dma_start(eye_sb[:, :], eye_in[:, :])
            ones_sb = const.tile([1, 128], f32r)
            nc.sync.dma_start(ones_sb[:, :], ones_in[:, :])

            # ---- px = pad_row @ Wx.T + bx ----
            px_ps = psum.tile([1, 1024], f32, tag="g")
            for nh in range(2):
                for k in range(3):
                    r = 128 if k < 2 else IN_DIM - 256
                    nc.tensor.matmul(
                        px_ps[:, nh * 512:(nh + 1) * 512],
                        padT_sb[0:r, k:k + 1],
                        WxT_sb[0:r, k * 1024 + nh * 512:k * 1024 + (nh + 1) * 512],
                        start=(k == 0), stop=(k == 2))
            px_sb = const.tile([1, 1024], f32)
            nc.vector.tensor_add(px_sb[:, :], px_ps[:, :], bx_sb[:, :])
            px_fm = const.tile([128, 8], f32)
            for m in range(8):
                tp = psum.tile([128, 1], f32, tag="tp", name=f"pxt{m}")
                nc.tensor.transpose(tp[:, :], px_sb[0:1, m * 128:(m + 1) * 128],
                                    eye_sb[0:1, 0:1])
                nc.scalar.copy(px_fm[:, m:m + 1], tp[:, :])
            px5 = const.tile([1, 1280], f32r)  # node-major 5-gate pad row
            for g in range(5):
                nc.vector.tensor_copy(
                    px5[0:1, 256 * g:256 * (g + 1)],
                    px_sb[0:1, _PX5SRC[g]:_PX5SRC[g] + 256])

            # ---- leaf phase ----
            c0 = state.tile([128, 2 * LPC], f32, tag="c")
            hev = state.tile([128, 2 * 512], f32r, tag="hev", name="hev_leaf")
            hod = state.tile([128, 2 * 512], f32r, tag="hod", name="hod_leaf")
            c0_3, hev3, hod3 = v2(c0), v2(hev), v2(hod)
            GL = 512
            for sg in range(LPC // GL):
                xg = {}
                for gname, gm in (("u", 0), ("i", 1), ("o", 3)):
                    t = psum.tile([128, 2 * GL], f32, tag="g", name=f"x{gname}{sg}")
                    for half in range(2):
                        m = gm * 2 + half
                        dst = t[:, half * GL:(half + 1) * GL]
                        for ki in range(3):
                            r = 128 if ki < 2 else IN_DIM - 256
                            nc.tensor.matmul(
                                dst,
                                WxT_sb[0:r, ki * 1024 + m * 128:
                                       ki * 1024 + (m + 1) * 128],
                                embsT_sb[0:r, ki * LPC + sg * GL:
                                         ki * LPC + (sg + 1) * GL],
                                start=(ki == 0), stop=(ki == 2))
                    xg[gname] = t
                ut = gates.tile([128, 2 * GL], f32, tag="u", name=f"u{sg}")
                it = gates.tile([128, 2 * GL], f32, tag="i", name=f"i{sg}")
                ot = gates.tile([128, 2 * GL], f32, tag="o", name=f"o{sg}")
                tht = gates.tile([128, 2 * GL], f32, tag="th", name=f"th{sg}")
                for gname, dst, fn, gm in (("u", ut, AF.Tanh, 0),
                                           ("i", it, AF.Sigmoid, 1),
                                           ("o", ot, AF.Sigmoid, 3)):
                    for half in range(2):
                        nc.scalar.activation(
                            dst[:, half * GL:(half + 1) * GL],
                            xg[gname][:, half * GL:(half + 1) * GL],
                            fn, bias=bx_fm[:, gm * 2 + half:gm * 2 + half + 1])
                cs = c0_3[:, :, sg * GL:(sg + 1) * GL]
                u3, i3, o3, th3 = v2(ut), v2(it), v2(ot), v2(tht)
                nc.vector.tensor_mul(cs, i3, u3)
                nc.scalar.activation(th3, cs, AF.Tanh)
                nc.vector.tensor_mul(hev3[:, :, sg * 256:(sg + 1) * 256],
                                     o3[:, :, 0::2], th3[:, :, 0::2])
                nc.vector.tensor_mul(hod3[:, :, sg * 256:(sg + 1) * 256],
                                     o3[:, :, 1::2], th3[:, :, 1::2])

            # ---- feature-major level (B >= 256) ----
            def fm_level(cp, hev_p, hod_p, Bp, lvl, split_c):
                B = Bp // 2
                hev_n = state.tile([128, 2 * (B // 2)], f32r, tag="hev",
                                   name=f"hev{lvl}")
                hod_n = state.tile([128, 2 * (B // 2)], f32r, tag="hod",
                                   name=f"hod{lvl}")
                if split_c:
                    cev = state.tile([128, 2 * (B // 2)], f32, tag="cev",
                                     name=f"cev{lvl}", bufs=1)
                    cod = state.tile([128, 2 * (B // 2)], f32, tag="cod",
                                     name=f"cod{lvl}", bufs=1)
                else:
                    cn = state.tile([128, 2 * B], f32, tag="c", name=f"c{lvl}")
                cp3 = v2(cp)
                for g0 in range(0, B, 256):
                    G = min(256, B - g0)
                    gt = []
                    for gi in range(5):
                        t = psum.tile([128, 2 * G], f32, tag="g",
                                      name=f"g{lvl}_{g0}_{gi}")
                        for half in range(2):
                            m = gi * 2 + half
                            dst = t[:, half * G:(half + 1) * G]
                            for ki in range(4):
                                W = WlT_sb if ki < 2 else WrT_sb
                                kc = ki % 2
                                hp = hev_p if ki < 2 else hod_p
                                nc.tensor.matmul(
                                    dst,
                                    W[:, kc * 1280 + m * 128:
                                      kc * 1280 + (m + 1) * 128],
                                    v2(hp)[:, kc, g0:g0 + G],
                                    start=(ki == 0), stop=(ki == 3))
                        gt.append(t)
                    sfx = f"{lvl}_{g0}"
                    ut = gates.tile([128, 2 * G], f32, tag="u", name=f"u{sfx}")
                    it = gates.tile([128, 2 * G], f32, tag="i", name=f"i{sfx}")
                    lft = gates.tile([128, 2 * G], f32, tag="lf", name=f"lf{sfx}")
                    rft = gates.tile([128, 2 * G], f32, tag="rf", name=f"rf{sfx}")
                    ot = gates.tile([128, 2 * G], f32, tag="o", name=f"o{sfx}")
                    tht = gates.tile([128, 2 * G], f32, tag="th", name=f"th{sfx}")
                    x1 = gates.tile([128, 2 * G], f32, tag="x1", name=f"x1{sfx}", bufs=1)
                    x2 = gates.tile([128, 2 * G], f32, tag="x2", name=f"x2{sfx}", bufs=1)
                    x3 = gates.tile([128, 2 * G], f32, tag="x3", name=f"x3{sfx}", bufs=1)
                    s1 = gates.tile([128, 2 * G], f32, tag="s1", name=f"s1{sfx}", bufs=1)
                    for gi, (dst, fn) in enumerate((
                            (ut, AF.Tanh), (it, AF.Sigmoid), (lft, AF.Sigmoid),
                            (rft, AF.Sigmoid), (ot, AF.Sigmoid))):
                        for half in range(2):
                            m = gi * 2 + half
                            nc.scalar.activation(
                                dst[:, half * G:(half + 1) * G],
                                gt[gi][:, half * G:(half + 1) * G],
                                fn, bias=px_fm[:, _PXCOL[m]:_PXCOL[m] + 1])
                    lc = cp3[:, :, 2 * g0:2 * (g0 + G):2]
                    rc = cp3[:, :, 2 * g0 + 1:2 * (g0 + G):2]
                    u3, i3 = v2(ut), v2(it)
                    lf3, rf3, o3, th3 = v2(lft), v2(rft), v2(ot), v2(tht)
                    x13, x23, x33, s13 = v2(x1), v2(x2), v2(x3), v2(s1)
                    nc.vector.tensor_mul(x13, i3, u3)
                    nc.vector.tensor_mul(x23, lf3, lc)
                    nc.vector.tensor_mul(x33, rf3, rc)
                    nc.vector.tensor_add(s13, x13, x23)
                    if split_c:
                        ce = v2(cev)[:, :, g0 // 2:(g0 + G) // 2]
                        co = v2(cod)[:, :, g0 // 2:(g0 + G) // 2]
                        nc.vector.tensor_add(ce, s13[:, :, 0::2], x33[:, :, 0::2])
                        nc.vector.tensor_add(co, s13[:, :, 1::2], x33[:, :, 1::2])
                        nc.scalar.activation(th3[:, :, 0::2], ce, AF.Tanh)
                        nc.scalar.activation(th3[:, :, 1::2], co, AF.Tanh)
                    else:
                        cs = v2(cn)[:, :, g0:g0 + G]
                        nc.vector.tensor_add(cs, s13, x33)
                        nc.scalar.activation(th3, cs, AF.Tanh)
                    nc.vector.tensor_mul(v2(hev_n)[:, :, g0 // 2:(g0 + G) // 2],
                                         o3[:, :, 0::2], th3[:, :, 0::2])
                    nc.vector.tensor_mul(v2(hod_n)[:, :, g0 // 2:(g0 + G) // 2],
                                         o3[:, :, 1::2], th3[:, :, 1::2])
                if split_c:
                    return (cev, cod), hev_n, hod_n, B
                return cn, hev_n, hod_n, B

            # ---- node-major level (B <= 128) ----
            # lcrc: [B, 512] tile, cols [0:256]=lc, [256:512]=rc
            def nm_level(lcrc, hev_p, hod_p, B, lvl, last, ntot=None, hoff=0,
                         tg=""):
                if ntot is None:
                    ntot = B
                g_ps = psum.tile([128, 1280], f32, tag="g", name=f"gn{lvl}{tg}")
                for n0, nw in ((0, 512), (512, 512), (1024, 256)):
                    for ki in range(5):
                        if ki < 4:
                            par, kc = ki // 2, ki % 2
                            hsrc = hev_p if par == 0 else hod_p
                            lhsT = hsrc[:, kc * ntot + hoff:kc * ntot + hoff + B]
                            W = WlT_sb if par == 0 else WrT_sb
                            rhs = W[:, kc * 1280 + n0:kc * 1280 + n0 + nw]
                        else:
                            lhsT = ones_sb[0:1, 0:B]
                            rhs = px5[0:1, n0:n0 + nw]
                        nc.tensor.matmul(g_ps[0:B, n0:n0 + nw], lhsT, rhs,
                                         start=(ki == 0), stop=(ki == 4))
                sfx = f"n{lvl}{tg}"
                ut = gates.tile([128, 256], f32, tag=f"u{tg}", name=f"u{sfx}", bufs=1)
                sig = gates.tile([128, 1024], f32, tag=f"sg{tg}", name=f"sg{sfx}", bufs=1)
                tht = gates.tile([128, 256], f32, tag=f"th{tg}", name=f"th{sfx}", bufs=1)
                x1 = gates.tile([128, 256], f32, tag=f"x1{tg}", name=f"x1{sfx}", bufs=1)
                x23 = gates.tile([128, 512], f32, tag=f"x23{tg}", name=f"x23{sfx}", bufs=1)
                s1 = gates.tile([128, 256], f32, tag=f"s1{tg}", name=f"s1{sfx}", bufs=1)
                c_nm = state.tile([128, 256], f32, tag=f"cn{tg}", name=f"cn{sfx}")
                h_nm = state.tile([128, 256], f32, tag=f"hn{tg}", name=f"hn{sfx}")
                nc.scalar.activation(ut[0:B, :], g_ps[0:B, 0:256], AF.Tanh)
                nc.scalar.activation(sig[0:B, :], g_ps[0:B, 256:1280], AF.Sigmoid)
                nc.vector.tensor_mul(x1[0:B, :], sig[0:B, 0:256], ut[0:B, :])
                nc.vector.tensor_mul(x23[0:B, :], sig[0:B, 256:768], lcrc[0:B, :])
                nc.vector.tensor_add(s1[0:B, :], x1[0:B, :], x23[0:B, 0:256])
                nc.vector.tensor_add(c_nm[0:B, :], s1[0:B, :], x23[0:B, 256:512])
                nc.scalar.activation(tht[0:B, :], c_nm[0:B, :], AF.Tanh)
                nc.vector.tensor_mul(h_nm[0:B, :], sig[0:B, 768:1024], tht[0:B, :])
                if last:
                    return c_nm, h_nm, None, None
                hev_n = state.tile([128, 2 * (B // 2)], f32r, tag=f"hev{tg}",
                                   name=f"hev{lvl}{tg}")
                hod_n = state.tile([128, 2 * (B // 2)], f32r, tag=f"hod{tg}",
                                   name=f"hod{lvl}{tg}")
                for kc in range(2):
                    tp = psum.tile([128, B], f32, tag="tp", name=f"tph{lvl}{tg}_{kc}")
                    nc.tensor.transpose(tp[:, :],
                                        h_nm[0:B, 128 * kc:128 * (kc + 1)],
                                        eye_sb[0:B, 0:B])
                    nc.vector.tensor_copy(
                        hev_n[:, kc * (B // 2):(kc + 1) * (B // 2)],
                        tp[:, 0:B:2])
                    nc.vector.tensor_copy(
                        hod_n[:, kc * (B // 2):(kc + 1) * (B // 2)],
                        tp[:, 1:B:2])
                return c_nm, h_nm, hev_n, hod_n

            def gather_children(c_src, B, lvl, tg=""):
                lcrc = gates.tile([128, 512], f32, tag=f"lcrc{tg}",
                                  name=f"lcrc{lvl}{tg}")
                nc.sync.dma_start(lcrc[0:B, 0:256], c_src[0:2 * B:2, :])
                nc.sync.dma_start(lcrc[0:B, 256:512], c_src[1:2 * B:2, :])
                return lcrc

            # lvl0 (1024->512, FM, contiguous c), lvl1 (512->256, FM, split c)
            c_lvl0, hev, hod, B = fm_level(c0, hev, hod, LPC, 0, False)
            (cev1, cod1), hev, hod, B = fm_level(c_lvl0, hev, hod, B, 1, True)

            # boundary: transpose split FM c into per-chain node-major lcrc
            lcrcA = gates.tile([128, 512], f32, tag="lcrcA", name="lcrcA2")
            lcrcB = gates.tile([128, 512], f32, tag="lcrcB", name="lcrcB2")
            for par, src in ((0, cev1), (1, cod1)):
                for kc in range(2):
                    tp = psum.tile([128, 128], f32, tag="tp",
                                   name=f"tpb{par}_{kc}")
                    nc.tensor.transpose(tp[:, :], v2(src)[:, kc, :],
                                        eye_sb[:, :])
                    cols = slice(256 * par + 128 * kc, 256 * par + 128 * (kc + 1))
                    nc.vector.tensor_copy(lcrcA[0:64, cols], tp[0:64, :])
                    stg = gates.tile([128, 128], f32, tag="stg",
                                     name=f"stg{par}_{kc}", bufs=1)
                    nc.vector.tensor_copy(stg[64:128, :], tp[64:128, :])
                    nc.sync.dma_start(lcrcB[0:64, cols], stg[64:128, :])

            # lvl2..lvl8: two independent half-subtree chains, interleaved
            chains = {
                "A": {"lcrc": lcrcA, "hev": hev, "hod": hod, "ntot": 128,
                      "hoff": 0},
                "B": {"lcrc": lcrcB, "hev": hev, "hod": hod, "ntot": 128,
                      "hoff": 64},
            }
            Bc = 64
            for lvl in range(2, 9):
                lastc = (lvl == 8)
                for tg in ("A", "B"):
                    ch = chains[tg]
                    c_nm, h_nm, hev_n, hod_n = nm_level(
                        ch["lcrc"], ch["hev"], ch["hod"], Bc, lvl, lastc,
                        ntot=ch["ntot"], hoff=ch["hoff"], tg=tg)
                    ch["c_nm"], ch["h_nm"] = c_nm, h_nm
                    if not lastc:
                        ch["hev"], ch["hod"] = hev_n, hod_n
                        ch["ntot"], ch["hoff"] = Bc // 2, 0
                        ch["lcrc"] = gather_children(c_nm, Bc // 2, lvl + 1, tg)
                Bc >>= 1

            # lvl9: merge the two chain roots (A = left/even, B = right/odd)
            lcrcM = gates.tile([128, 512], f32, tag="lcrcA", name="lcrcM")
            nc.sync.dma_start(lcrcM[0:1, 0:256], chains["A"]["c_nm"][0:1, :])
            nc.sync.dma_start(lcrcM[0:1, 256:512], chains["B"]["c_nm"][0:1, :])
            hevM = state.tile([128, 2], f32r, tag="hevA", name="hevM")
            hodM = state.tile([128, 2], f32r, tag="hodA", name="hodM")
            for tg, dst in (("A", hevM), ("B", hodM)):
                hroot = chains[tg]["h_nm"]
                for kc in range(2):
                    tp = psum.tile([128, 1], f32, tag="tp", name=f"tpm{tg}{kc}")
                    nc.tensor.transpose(tp[:, :], hroot[0:1, 128 * kc:128 * (kc + 1)],
                                        eye_sb[0:1, 0:1])
                    nc.vector.tensor_copy(dst[:, kc:kc + 1], tp[:, :])
            c_nm, h_nm, _, _ = nm_level(lcrcM, hevM, hodM, 1, 9, True, ntot=1,
                                        tg="A")

            # ---- AllGather the 8 per-core roots ----
            cc_in = dram.tile([1, 512], f32)
            cc_out = dram.tile([8, 512], f32, addr_space="Shared")
            nc.sync.dma_start(cc_in[0:1, 0:256], c_nm[0:1, :])
            nc.sync.dma_start(cc_in[0:1, 256:512], h_nm[0:1, :])
            nc.gpsimd.collective_compute(
                "AllGather",
                mybir.AluOpType.bypass,
                replica_groups=[list(range(N_CORES))],
                ins=[cc_in.opt()],
                outs=[cc_out.opt()],
            )
            roots_sb = const.tile([8, 512], f32)
            nc.sync.dma_start(roots_sb[:, :], cc_out[:, :])

            # prep lvl10 inputs from the gathered roots
            hev = state.tile([128, 2 * 4], f32r, tag="hevA", name="hev_ag")
            hod = state.tile([128, 2 * 4], f32r, tag="hodA", name="hod_ag")
            for kc in range(2):
                tp = psum.tile([128, 8], f32, tag="tp", name=f"tpag{kc}")
                nc.tensor.transpose(
                    tp[:, :], roots_sb[0:8, 256 + 128 * kc:256 + 128 * (kc + 1)],
                    eye_sb[0:8, 0:8])
                nc.vector.tensor_copy(hev[:, kc * 4:(kc + 1) * 4], tp[:, 0:8:2])
                nc.vector.tensor_copy(hod[:, kc * 4:(kc + 1) * 4], tp[:, 1:8:2])
            lcrc = gates.tile([128, 512], f32, tag="lcrcA", name="lcrc10")
            nc.sync.dma_start(lcrc[0:4, 0:256], roots_sb[0:8:2, 0:256])
            nc.sync.dma_start(lcrc[0:4, 256:512], roots_sb[1:8:2, 0:256])

            # final 3 levels, replicated (B = 4, 2, 1)
            B = 8
            for lvl in range(10, 13):
                B >>= 1
                last = (lvl == 12)
                c_nm, h_nm, hev_n, hod_n = nm_level(lcrc, hev, hod, B, lvl,
                                                    last, tg="A")
                if not last:
                    hev, hod = hev_n, hod_n
                    lcrc = gather_children(c_nm, B // 2, lvl + 1, "A")

            # ---- write root (c, h) ----
            nc.sync.dma_start(out[0:1, 0:1, :], c_nm[0:1, :])
            nc.sync.dma_start(out[1:2, 0:1, :], h_nm[0:1, :])

    nc.compile()
    return nc


def _get_nc():
    if "nc" not in _CACHE:
        _CACHE["nc"] = _build()
    return _CACHE["nc"]


def kernel(embs, Wx, bx, Wl, Wr, emb_table, _trace=False, _trace_kwargs=None):
    from concourse.bass_utils import run_bass_kernel_spmd

    embs = np.ascontiguousarray(np.asarray(embs, dtype=np.float32))
    Wx = np.asarray(Wx, dtype=np.float32)
    bx = np.asarray(bx, dtype=np.float32)
    Wl = np.asarray(Wl, dtype=np.float32)
    Wr = np.asarray(Wr, dtype=np.float32)
    emb_table = np.asarray(emb_table, dtype=np.float32)

    WxT = np.ascontiguousarray(Wx.T)
    WlT = np.ascontiguousarray(Wl.T)
    WrT = np.ascontiguousarray(Wr.T)
    bxr = np.ascontiguousarray(bx.reshape(1, 1024))
    padT = np.ascontiguousarray(emb_table[-1].reshape(IN_DIM, 1))
    eye = np.eye(128, dtype=np.float32)
    ones = np.ones((1, 128), dtype=np.float32)

    in_maps = []
    for d in range(N_CORES):
        shard = np.ascontiguousarray(embs[d * LPC:(d + 1) * LPC].T)
        in_maps.append({
            "embsT": shard, "WxT": WxT, "WlT": WlT, "WrT": WrT,
            "bxr": bxr, "padT": padT, "eye_in": eye, "ones_in": ones,
        })

    nc = _get_nc()
    res = run_bass_kernel_spmd(nc, in_maps, list(range(N_CORES)),
                               trace=_trace, **(_trace_kwargs or {}))
    _CACHE["last_result"] = res
    return np.asarray(res.results[0]["out"], dtype=np.float32)


# revision 17
# speedup vs baseline: 1.1813x; 1.1813x over previous
"""BinaryTreeLSTM on 8 Trainium2 NeuronCores.

Data-parallel over the leaf batch: core d owns leaves [1024d, 1024d+1024)
and folds its subtree through 10 merge levels; the 8 per-core roots are
AllGathered and the final 3 levels run replicated on every core.

Two matmul regimes (fp32r operands, single-pass PE):
- Feature-major (leaf, B=512, B=256 levels): weights stationary, nodes
  on the moving free dim. State h is kept as [128, 2 chunks * B] with
  even/odd children split into separate tiles so weight loads and reads
  stay contiguous.
- Node-major (B <= 128 levels): h chunks stationary (tiny weight loads),
  W streams as the moving operand in 512-wide chunks. Gates/c/h are
  node-major [B, 256]; h is transposed back to feature-major via PE
  transposes for the next level, and lc/rc come from partition-strided
  SBUF DMAs of the previous node-major c.
"""

import numpy as np

IN_DIM = 300
MEM_DIM = 256
N_LEAVES = 8192
N_CORES = 8
LPC = N_LEAVES // N_CORES  # 1024 leaves per core

# FM-gate m-chunk (5-gate [u,i,lf,rf,o] x 2 halves) -> column of the
# [128, 8] feature-major pad_xg ([cx,ix,fx,ox]; lf and rf share fx)
_PXCOL = [0, 1, 2, 3, 4, 5, 4, 5, 6, 7]
# node-major 5-gate px layout offsets into the 4-gate [1,1024] px row
_PX5SRC = [0, 256, 512, 512, 768]

_CACHE = {}


def _build():
    import concourse.bacc as bacc
    import concourse.mybir as mybir
    import concourse.tile as tile

    f32 = mybir.dt.float32
    f32r = mybir.dt.float32r
    AF = mybir.ActivationFunctionType

    nc = bacc.Bacc("TRN2", target_bir_lowering=False, debug=False,
                   num_devices=N_CORES)

    embsT = nc.dram_tensor("embsT", [IN_DIM, LPC], f32r, kind="ExternalInput").ap()
    WxT = nc.dram_tensor("WxT", [IN_DIM, 1024], f32r, kind="ExternalInput").ap()
    WlT = nc.dram_tensor("WlT", [MEM_DIM, 1280], f32r, kind="ExternalInput").ap()
    WrT = nc.dram_tensor("WrT", [MEM_DIM, 1280], f32r, kind="ExternalInput").ap()
    bxr = nc.dram_tensor("bxr", [1, 1024], f32, kind="ExternalInput").ap()
    padT = nc.dram_tensor("padT", [IN_DIM, 1], f32r, kind="ExternalInput").ap()
    eye_in = nc.dram_tensor("eye_in", [128, 128], f32, kind="ExternalInput").ap()
    ones_in = nc.dram_tensor("ones_in", [1, 128], f32r, kind="ExternalInput").ap()
    out = nc.dram_tensor("out", [2, 1, MEM_DIM], f32, kind="ExternalOutput").ap()

    with tile.TileContext(nc) as tc:
        with (
            tc.tile_pool(name="const", bufs=1) as const,
            tc.tile_pool(name="state", bufs=2) as state,
            tc.tile_pool(name="gates", bufs=2) as gates,
            tc.tile_pool(name="psum", bufs=2, space="PSUM") as psum,
            tc.tile_pool(name="dram", bufs=1, space="DRAM") as dram,
        ):
            v2 = lambda t: t.rearrange("p (c n) -> p c n", c=2)

            # ---- constants ----
            WxT_sb = const.tile([128, 3 * 1024], f32r)
            embsT_sb = const.tile([128, 3 * LPC], f32r)
            for k in range(3):
                r = 128 if k < 2 else IN_DIM - 256
                nc.sync.dma_start(WxT_sb[0:r, k * 1024:(k + 1) * 1024],
                                  WxT[128 * k:128 * k + r, :])
                nc.sync.dma_start(embsT_sb[0:r, k * LPC:(k + 1) * LPC],
                                  embsT[128 * k:128 * k + r, :])
            WlT_sb = const.tile([128, 2 * 1280], f32r)
            WrT_sb = const.tile([128, 2 * 1280], f32r)
            for k in range(2):
                nc.sync.dma_start(WlT_sb[:, k * 1280:(k + 1) * 1280],
                                  WlT[128 * k:128 * (k + 1), :])
                nc.sync.dma_start(WrT_sb[:, k * 1280:(k + 1) * 1280],
                                  WrT[128 * k:128 * (k + 1), :])
            bx_sb = const.tile([1, 1024], f32)
            nc.sync.dma_start(bx_sb[:, :], bxr[:, :])
            bx_fm = const.tile([128, 8], f32)
            nc.sync.dma_start(bx_fm[:, :],
                              bxr.rearrange("o (m p) -> p (o m)", p=128))
            padT_sb = const.tile([128, 3], f32r)
            for k in range(3):
                r = 128 if k < 2 else IN_DIM - 256
                nc.sync.dma_start(padT_sb[0:r, k:k + 1], padT[128 * k:128 * k + r, :])
            eye_sb = const.tile([128, 128], f32)
            nc.sync.dma_start(eye_sb[:, :], eye_in[:, :])
            ones_sb = const.tile([1, 128], f32r)
            nc.sync.dma_start(ones_sb[:, :], ones_in[:, :])

            # ---- px = pad_row @ Wx.T + bx ----
            px_ps = psum.tile([1, 1024], f32, tag="g")
            for nh in range(2):
                for k in range(3):
                    r = 128 if k < 2 else IN_DIM - 256
                    nc.tensor.matmul(
                        px_ps[:, nh * 512:(nh + 1) * 512],
                        padT_sb[0:r, k:k + 1],
                        WxT_sb[0:r, k * 1024 + nh * 512:k * 1024 + (nh + 1) * 512],
                        start=(k == 0), stop=(k == 2))
            px_sb = const.tile([1, 1024], f32)
            nc.vector.tensor_add(px_sb[:, :], px_ps[:, :], bx_sb[:, :])
            px_fm = const.tile([128, 8], f32)
            for m in range(8):
                tp = psum.tile([128, 1], f32, tag="tp", name=f"pxt{m}")
                nc.tensor.transpose(tp[:, :], px_sb[0:1, m * 128:(m + 1) * 128],
                                    eye_sb[0:1, 0:1])
                nc.scalar.copy(px_fm[:, m:m + 1], tp[:, :])
            px5 = const.tile([1, 1280], f32r)  # node-major 5-gate pad row
            for g in range(5):
                nc.vector.tensor_copy(
                    px5[0:1, 256 * g:256 * (g + 1)],
                    px_sb[0:1, _PX5SRC[g]:_PX5SRC[g] + 256])

            # ---- leaf phase ----
            c0 = state.tile([128, 2 * LPC], f32, tag="c")
            hev = state.tile([128, 2 * 512], f32r, tag="hev", name="hev_leaf")
            hod = state.tile([128, 2 * 512], f32r, tag="hod", name="hod_leaf")
            c0_3, hev3, hod3 = v2(c0), v2(hev), v2(hod)
            GL = 512
            for sg in range(LPC // GL):
                xg = {}
                for gname, gm in (("u", 0), ("i", 1), ("o", 3)):
                    t = psum.tile([128, 2 * GL], f32, tag="g", name=f"x{gname}{sg}")
                    for half in range(2):
                        m = gm * 2 + half
                        dst = t[:, half * GL:(half + 1) * GL]
                        for ki in range(3):
                            r = 128 if ki < 2 else IN_DIM - 256
                            nc.tensor.matmul(
                                dst,
                                WxT_sb[0:r, ki * 1024 + m * 128:
                                       ki * 1024 + (m + 1) * 128],
                                embsT_sb[0:r, ki * LPC + sg * GL:
                                         ki * LPC + (sg + 1) * GL],
                                start=(ki == 0), stop=(ki == 2))
                    xg[gname] = t
                ut = gates.tile([128, 2 * GL], f32, tag="u", name=f"u{sg}")
                it = gates.tile([128, 2 * GL], f32, tag="i", name=f"i{sg}")
                ot = gates.tile([128, 2 * GL], f32, tag="o", name=f"o{sg}")
                tht = gates.tile([128, 2 * GL], f32, tag="th", name=f"th{sg}")
                for gname, dst, fn, gm in (("u", ut, AF.Tanh, 0),
                                           ("i", it, AF.Sigmoid, 1),
                                           ("o", ot, AF.Sigmoid, 3)):
                    for half in range(2):
                        nc.scalar.activation(
                            dst[:, half * GL:(half + 1) * GL],
                            xg[gname][:, half * GL:(half + 1) * GL],
                            fn, bias=bx_fm[:, gm * 2 + half:gm * 2 + half + 1])
                cs = c0_3[:, :, sg * GL:(sg + 1) * GL]
                u3, i3, o3, th3 = v2(ut), v2(it), v2(ot), v2(tht)
                nc.vector.tensor_mul(cs, i3, u3)
                nc.scalar.activation(th3, cs, AF.Tanh)
                nc.vector.tensor_mul(hev3[:, :, sg * 256:(sg + 1) * 256],
                                     o3[:, :, 0::2], th3[:, :, 0::2])
                nc.vector.tensor_mul(hod3[:, :, sg * 256:(sg + 1) * 256],
                                     o3[:, :, 1::2], th3[:, :, 1::2])

            # ---- feature-major level (B >= 256) ----
            def fm_level(cp, hev_p, hod_p, Bp, lvl, split_c):
                B = Bp // 2
                hev_n = state.tile([128, 2 * (B // 2)], f32r, tag="hev",
                                   name=f"hev{lvl}")
                hod_n = state.tile([128, 2 * (B // 2)], f32r, tag="hod",
                                   name=f"hod{lvl}")
                if split_c:
                    cev = state.tile([128, 2 * (B // 2)], f32, tag="cev",
                                     name=f"cev{lvl}", bufs=1)
                    cod = state.tile([128, 2 * (B // 2)], f32, tag="cod",
                                     name=f"cod{lvl}", bufs=1)
                else:
                    cn = state.tile([128, 2 * B], f32, tag="c", name=f"c{lvl}")
                cp3 = v2(cp)
                for g0 in range(0, B, 256):
                    G = min(256, B - g0)
                    gt = []
                    for gi in range(5):
                        t = psum.tile([128, 2 * G], f32, tag="g",
                                      name=f"g{lvl}_{g0}_{gi}")
                        for half in range(2):
                            m = gi * 2 + half
                            dst = t[:, half * G:(half + 1) * G]
                            for ki in range(4):
                                W = WlT_sb if ki < 2 else WrT_sb
                                kc = ki % 2
                                hp = hev_p if ki < 2 else hod_p
                                nc.tensor.matmul(
                                    dst,
                                    W[:, kc * 1280 + m * 128:
                                      kc * 1280 + (m + 1) * 128],
                                    v2(hp)[:, kc, g0:g0 + G],
                                    start=(ki == 0), stop=(ki == 3))
                        gt.append(t)
                    sfx = f"{lvl}_{g0}"
                    ut = gates.tile([128, 2 * G], f32, tag="u", name=f"u{sfx}")
                    it = gates.tile([128, 2 * G], f32, tag="i", name=f"i{sfx}")
                    lft = gates.tile([128, 2 * G], f32, tag="lf", name=f"lf{sfx}")
                    rft = gates.tile([128, 2 * G], f32, tag="rf", name=f"rf{sfx}")
                    ot = gates.tile([128, 2 * G], f32, tag="o", name=f"o{sfx}")
                    tht = gates.tile([128, 2 * G], f32, tag="th", name=f"th{sfx}")
                    x1 = gates.tile([128, 2 * G], f32, tag="x1", name=f"x1{sfx}", bufs=1)
                    x2 = gates.tile([128, 2 * G], f32, tag="x2", name=f"x2{sfx}", bufs=1)
                    x3 = gates.tile([128, 2 * G], f32, tag="x3", name=f"x3{sfx}", bufs=1)
                    s1 = gates.tile([128, 2 * G], f32, tag="s1", name=f"s1{sfx}", bufs=1)
                    for gi, (dst, fn) in enumerate((
                            (ut, AF.Tanh), (it, AF.Sigmoid), (lft, AF.Sigmoid),
                            (rft, AF.Sigmoid), (ot, AF.Sigmoid))):
                        for half in range(2):
                            m = gi * 2 + half
                            nc.scalar.activation(
                                dst[:, half * G:(half + 1) * G],
                                gt[gi][:, half * G:(half + 1) * G],
                                fn, bias=px_fm[:, _PXCOL[m]:_PXCOL[m] + 1])
                    lc = cp3[:, :, 2 * g0:2 * (g0 + G):2]
                    rc = cp3[:, :, 2 * g0 + 1:2 * (g0 + G):2]
                    u3, i3 = v2(ut), v2(it)
                    lf3, rf3, o3, th3 = v2(lft), v2(rft), v2(ot), v2(tht)
                    x13, x23, x33, s13 = v2(x1), v2(x2), v2(x3), v2(s1)
                    nc.vector.tensor_mul(x13, i3, u3)
                    nc.vector.tensor_mul(x23, lf3, lc)
                    nc.vector.tensor_mul(x33, rf3, rc)
                    nc.vector.tensor_add(s13, x13, x23)
                    if split_c:
                        ce = v2(cev)[:, :, g0 // 2:(g0 + G) // 2]
                        co = v2(cod)[:, :, g0 // 2:(g0 + G) // 2]
                        nc.vector.tensor_add(ce, s13[:, :, 0::2], x33[:, :, 0::2])
                        nc.vector.tensor_add(co, s13[:, :, 1::2], x33[:, :, 1::2])
                        nc.scalar.activation(th3[:, :, 0::2], ce, AF.Tanh)
                        nc.scalar.activation(th3[:, :, 1::2], co, AF.Tanh)
                    else:
                        cs = v2(cn)[:, :, g0:g0 + G]
                        nc.vector.tensor_add(cs, s13, x33)
                        nc.scalar.activation(th3, cs, AF.Tanh)
                    nc.vector.tensor_mul(v2(hev_n)[:, :, g0 // 2:(g0 + G) // 2],
                                         o3[:, :, 0::2], th3[:, :, 0::2])
                    nc.vector.tensor_mul(v2(hod_n)[:, :, g0 // 2:(g0 + G) // 2],
                                         o3[:, :, 1::2], th3[:, :, 1::2])
                if split_c:
                    return (cev, cod), hev_n, hod_n, B
                return cn, hev_n, hod_n, B

            # ---- node-major level (B <= 128) ----
            # lcrc: [B, 512] tile, cols [0:256]=lc, [256:512]=rc
            def nm_level(lcrc, hev_p, hod_p, B, lvl, last, ntot=None, hoff=0,
                         tg=""):
                if ntot is None:
                    ntot = B
                g_ps = psum.tile([128, 1280], f32, tag="g", name=f"gn{lvl}{tg}")
                for ki in range(5):
                    for n0, nw in ((0, 512), (512, 512), (1024, 256)):
                        if ki < 4:
                            par, kc = ki // 2, ki % 2
                            hsrc = hev_p if par == 0 else hod_p
                            lhsT = hsrc[:, kc * ntot + hoff:kc * ntot + hoff + B]
                            W = WlT_sb if par == 0 else WrT_sb
                            rhs = W[:, kc * 1280 + n0:kc * 1280 + n0 + nw]
                        else:
                            lhsT = ones_sb[0:1, 0:B]
                            rhs = px5[0:1, n0:n0 + nw]
                        nc.tensor.matmul(g_ps[0:B, n0:n0 + nw], lhsT, rhs,
                                         start=(ki == 0), stop=(ki == 4))
                sfx = f"n{lvl}{tg}"
                ut = gates.tile([128, 256], f32, tag=f"u{tg}", name=f"u{sfx}", bufs=1)
                sig = gates.tile([128, 1024], f32, tag=f"sg{tg}", name=f"sg{sfx}", bufs=1)
                tht = gates.tile([128, 256], f32, tag=f"th{tg}", name=f"th{sfx}", bufs=1)
                x1 = gates.tile([128, 256], f32, tag=f"x1{tg}", name=f"x1{sfx}", bufs=1)
                x23 = gates.tile([128, 512], f32, tag=f"x23{tg}", name=f"x23{sfx}", bufs=1)
                s1 = gates.tile([128, 256], f32, tag=f"s1{tg}", name=f"s1{sfx}", bufs=1)
                c_nm = state.tile([128, 256], f32, tag=f"cn{tg}", name=f"cn{sfx}")
                h_nm = state.tile([128, 256], f32, tag=f"hn{tg}", name=f"hn{sfx}")
                nc.scalar.activation(ut[0:B, :], g_ps[0:B, 0:256], AF.Tanh)
                nc.scalar.activation(sig[0:B, :], g_ps[0:B, 256:1280], AF.Sigmoid)
                nc.vector.tensor_mul(x1[0:B, :], sig[0:B, 0:256], ut[0:B, :])
                nc.vector.tensor_mul(x23[0:B, :], sig[0:B, 256:768], lcrc[0:B, :])
                nc.vector.tensor_add(s1[0:B, :], x1[0:B, :], x23[0:B, 0:256])
                nc.vector.tensor_add(c_nm[0:B, :], s1[0:B, :], x23[0:B, 256:512])
                nc.scalar.activation(tht[0:B, :], c_nm[0:B, :], AF.Tanh)
                nc.vector.tensor_mul(h_nm[0:B, :], sig[0:B, 768:1024], tht[0:B, :])
                if last:
                    return c_nm, h_nm, None, None
                hev_n = state.tile([128, 2 * (B // 2)], f32r, tag=f"hev{tg}",
                                   name=f"hev{lvl}{tg}")
                hod_n = state.tile([128, 2 * (B // 2)], f32r, tag=f"hod{tg}",
                                   name=f"hod{lvl}{tg}")
                for kc in range(2):
                    tp = psum.tile([128, B], f32, tag="tp", name=f"tph{lvl}{tg}_{kc}")
                    nc.tensor.transpose(tp[:, :],
                                        h_nm[0:B, 128 * kc:128 * (kc + 1)],
                                        eye_sb[0:B, 0:B])
                    nc.vector.tensor_copy(
                        hev_n[:, kc * (B // 2):(kc + 1) * (B // 2)],
                        tp[:, 0:B:2])
                    nc.vector.tensor_copy(
                        hod_n[:, kc * (B // 2):(kc + 1) * (B // 2)],
                        tp[:, 1:B:2])
                return c_nm, h_nm, hev_n, hod_n

            def gather_children(c_src, B, lvl, tg=""):
                lcrc = gates.tile([128, 512], f32, tag=f"lcrc{tg}",
                                  name=f"lcrc{lvl}{tg}")
                nc.sync.dma_start(lcrc[0:B, 0:256], c_src[0:2 * B:2, :])
                nc.sync.dma_start(lcrc[0:B, 256:512], c_src[1:2 * B:2, :])
                return lcrc

            # lvl0 (1024->512, FM, contiguous c), lvl1 (512->256, FM, split c)
            c_lvl0, hev, hod, B = fm_level(c0, hev, hod, LPC, 0, False)
            (cev1, cod1), hev, hod, B = fm_level(c_lvl0, hev, hod, B, 1, True)

            # boundary: transpose split FM c into node-major lcrc for lvl2
            lcrc = gates.tile([128, 512], f32, tag="lcrcA", name="lcrc2")
            for par, src in ((0, cev1), (1, cod1)):
                for kc in range(2):
                    tp = psum.tile([128, 128], f32, tag="tp",
                                   name=f"tpb{par}_{kc}")
                    nc.tensor.transpose(tp[:, :], v2(src)[:, kc, :],
                                        eye_sb[:, :])
                    nc.vector.tensor_copy(
                        lcrc[:, 256 * par + 128 * kc:256 * par + 128 * (kc + 1)],
                        tp[:, :])

            # lvl2..lvl9 node-major (B = 128..1)
            hevp, hodp, ntot = hev, hod, 128
            for lvl in range(2, 10):
                B >>= 1  # 128, 64, ..., 1
                last = (lvl == 9)
                c_nm, h_nm, hev_n, hod_n = nm_level(lcrc, hevp, hodp, B, lvl,
                                                    last, ntot=ntot, tg="A")
                if not last:
                    hevp, hodp, ntot = hev_n, hod_n, B // 2
                    lcrc = gather_children(c_nm, B // 2, lvl + 1, "A")

            # ---- AllGather the 8 per-core roots ----
            cc_in = dram.tile([1, 512], f32)
            cc_out = dram.tile([8, 512], f32, addr_space="Shared")
            nc.sync.dma_start(cc_in[0:1, 0:256], c_nm[0:1, :])
            nc.sync.dma_start(cc_in[0:1, 256:512], h_nm[0:1, :])
            nc.gpsimd.collective_compute(
                "AllGather",
                mybir.AluOpType.bypass,
                replica_groups=[list(range(N_CORES))],
                ins=[cc_in.opt()],
                outs=[cc_out.opt()],
            )
            roots_sb = const.tile([8, 512], f32)
            nc.sync.dma_start(roots_sb[:, :], cc_out[:, :])

            # prep lvl10 inputs from the gathered roots
            hev = state.tile([128, 2 * 4], f32r, tag="hevA", name="hev_ag")
            hod = state.tile([128, 2 * 4], f32r, tag="hodA", name="hod_ag")
            for kc in range(2):
                tp = psum.tile([128, 8], f32, tag="tp", name=f"tpag{kc}")
                nc.tensor.transpose(
                    tp[:, :], roots_sb[0:8, 256 + 128 * kc:256 + 128 * (kc + 1)],
                    eye_sb[0:8, 0:8])
                nc.vector.tensor_copy(hev[:, kc * 4:(kc + 1) * 4], tp[:, 0:8:2])
                nc.vector.tensor_copy(hod[:, kc * 4:(kc + 1) * 4], tp[:, 1:8:2])
            lcrc = gates.tile([128, 512], f32, tag="lcrcA", name="lcrc10")
            nc.sync.dma_start(lcrc[0:4, 0:256], roots_sb[0:8:2, 0:256])
            nc.sync.dma_start(lcrc[0:4, 256:512], roots_sb[1:8:2, 0:256])

            # final 3 levels, replicated (B = 4, 2, 1)
            B = 8
            for lvl in range(10, 13):
                B >>= 1
                last = (lvl == 12)
                c_nm, h_nm, hev_n, hod_n = nm_level(lcrc, hev, hod, B, lvl,
                                                    last, tg="A")
                if not last:
                    hev, hod = hev_n, hod_n
                    lcrc = gather_children(c_nm, B // 2, lvl + 1, "A")

            # ---- write root (c, h) ----
            nc.sync.dma_start(out[0:1, 0:1, :], c_nm[0:1, :])
            nc.sync.dma_start(out[1:2, 0:1, :], h_nm[0:1, :])

    nc.compile()
    return nc


def _get_nc():
    if "nc" not in _CACHE:
        _CACHE["nc"] = _build()
    return _CACHE["nc"]


def kernel(embs, Wx, bx, Wl, Wr, emb_table, _trace=False, _trace_kwargs=None):
    from concourse.bass_utils import run_bass_kernel_spmd

    embs = np.ascontiguousarray(np.asarray(embs, dtype=np.float32))
    Wx = np.asarray(Wx, dtype=np.float32)
    bx = np.asarray(bx, dtype=np.float32)
    Wl = np.asarray(Wl, dtype=np.float32)
    Wr = np.asarray(Wr, dtype=np.float32)
    emb_table = np.asarray(emb_table, dtype=np.float32)

    WxT = np.ascontiguousarray(Wx.T)
    WlT = np.ascontiguousarray(Wl.T)
    WrT = np.ascontiguousarray(Wr.T)
    bxr = np.ascontiguousarray(bx.reshape(1, 1024))
    padT = np.ascontiguousarray(emb_table[-1].reshape(IN_DIM, 1))
    eye = np.eye(128, dtype=np.float32)
    ones = np.ones((1, 128), dtype=np.float32)

    in_maps = []
    for d in range(N_CORES):
        shard = np.ascontiguousarray(embs[d * LPC:(d + 1) * LPC].T)
        in_maps.append({
            "embsT": shard, "WxT": WxT, "WlT": WlT, "WrT": WrT,
            "bxr": bxr, "padT": padT, "eye_in": eye, "ones_in": ones,
        })

    nc = _get_nc()
    res = run_bass_kernel_spmd(nc, in_maps, list(range(N_CORES)),
                               trace=_trace, **(_trace_kwargs or {}))
    _CACHE["last_result"] = res
    return np.asarray(res.results[0]["out"], dtype=np.float32)


# revision 18
# speedup vs baseline: 1.6495x; 1.3964x over previous
"""BinaryTreeLSTM on 8 Trainium2 NeuronCores.

Data-parallel over the leaf batch: core d owns leaves [1024d, 1024d+1024)
and folds its subtree through 10 merge levels; the 8 per-core roots are
AllGathered and the final 3 levels run replicated on every core.

Two matmul regimes (fp32r operands, single-pass PE):
- Feature-major (leaf, B=512, B=256 levels): weights stationary, nodes
  on the moving free dim. State h is kept as [128, 2 chunks * B] with
  even/odd children split into separate tiles so weight loads and reads
  stay contiguous.
- Node-major (B <= 128 levels): h chunks stationary (tiny weight loads),
  W streams as the moving operand in 512-wide chunks. Gates/c/h are
  node-major [B, 256]; h is transposed back to feature-major via PE
  transposes for the next level, and lc/rc come from partition-strided
  SBUF DMAs of the previous node-major c.
"""

import numpy as np

IN_DIM = 300
MEM_DIM = 256
N_LEAVES = 8192
N_CORES = 8
LPC = N_LEAVES // N_CORES  # 1024 leaves per core

# FM-gate m-chunk (5-gate [u,i,lf,rf,o] x 2 halves) -> column of the
# [128, 8] feature-major pad_xg ([cx,ix,fx,ox]; lf and rf share fx)
_PXCOL = [0, 1, 2, 3, 4, 5, 4, 5, 6, 7]
# node-major 5-gate px layout offsets into the 4-gate [1,1024] px row
_PX5SRC = [0, 256, 512, 512, 768]

_CACHE = {}


def _build():
    import concourse.bacc as bacc
    import concourse.mybir as mybir
    import concourse.tile as tile

    f32 = mybir.dt.float32
    f32r = mybir.dt.float32r
    AF = mybir.ActivationFunctionType

    nc = bacc.Bacc("TRN2", target_bir_lowering=False, debug=False,
                   num_devices=N_CORES)

    embsT = nc.dram_tensor("embsT", [IN_DIM, LPC], f32r, kind="ExternalInput").ap()
    WxT = nc.dram_tensor("WxT", [IN_DIM, 1024], f32r, kind="ExternalInput").ap()
    WlT = nc.dram_tensor("WlT", [MEM_DIM, 1280], f32r, kind="ExternalInput").ap()
    WrT = nc.dram_tensor("WrT", [MEM_DIM, 1280], f32r, kind="ExternalInput").ap()
    bxr = nc.dram_tensor("bxr", [1, 1024], f32, kind="ExternalInput").ap()
    padT = nc.dram_tensor("padT", [IN_DIM, 1], f32r, kind="ExternalInput").ap()
    eye_in = nc.dram_tensor("eye_in", [128, 128], f32, kind="ExternalInput").ap()
    ones_in = nc.dram_tensor("ones_in", [1, 128], f32r, kind="ExternalInput").ap()
    out = nc.dram_tensor("out", [2, MEM_DIM], f32, kind="ExternalOutput").ap()

    with tile.TileContext(nc) as tc:
        with (
            tc.tile_pool(name="const", bufs=1) as const,
            tc.tile_pool(name="state", bufs=2) as state,
            tc.tile_pool(name="gates", bufs=2) as gates,
            tc.tile_pool(name="psum", bufs=2, space="PSUM") as psum,
            tc.tile_pool(name="dram", bufs=1, space="DRAM") as dram,
        ):
            v2 = lambda t: t.rearrange("p (c n) -> p c n", c=2)

            # ---- constants ----
            WxT_sb = const.tile([128, 3 * 1024], f32r)
            embsT_sb = const.tile([128, 3 * LPC], f32r)
            for k in range(3):
                r = 128 if k < 2 else IN_DIM - 256
                nc.sync.dma_start(WxT_sb[0:r, k * 1024:(k + 1) * 1024],
                                  WxT[128 * k:128 * k + r, :])
                nc.sync.dma_start(embsT_sb[0:r, k * LPC:(k + 1) * LPC],
                                  embsT[128 * k:128 * k + r, :])
            WlT_sb = const.tile([128, 2 * 1280], f32r)
            WrT_sb = const.tile([128, 2 * 1280], f32r)
            for k in range(2):
                nc.sync.dma_start(WlT_sb[:, k * 1280:(k + 1) * 1280],
                                  WlT[128 * k:128 * (k + 1), :])
                nc.sync.dma_start(WrT_sb[:, k * 1280:(k + 1) * 1280],
                                  WrT[128 * k:128 * (k + 1), :])
            bx_sb = const.tile([1, 1024], f32)
            nc.sync.dma_start(bx_sb[:, :], bxr[:, :])
            bx_fm = const.tile([128, 8], f32)
            nc.sync.dma_start(bx_fm[:, :],
                              bxr.rearrange("o (m p) -> p (o m)", p=128))
            padT_sb = const.tile([128, 3], f32r)
            for k in range(3):
                r = 128 if k < 2 else IN_DIM - 256
                nc.sync.dma_start(padT_sb[0:r, k:k + 1], padT[128 * k:128 * k + r, :])
            eye_sb = const.tile([128, 128], f32)
            nc.sync.dma_start(eye_sb[:, :], eye_in[:, :])
            ones_sb = const.tile([1, 128], f32r)
            nc.sync.dma_start(ones_sb[:, :], ones_in[:, :])

            # ---- leaf phase ----
            c0 = state.tile([128, 2 * LPC], f32, tag="c")
            hev = state.tile([128, 2 * 512], f32r, tag="hev", name="hev_leaf")
            hod = state.tile([128, 2 * 512], f32r, tag="hod", name="hod_leaf")
            c0_3, hev3, hod3 = v2(c0), v2(hev), v2(hod)
            GL = 512
            for sg in range(LPC // GL):
                xg = {}
                for gname, gm in (("u", 0), ("i", 1), ("o", 3)):
                    t = psum.tile([128, 2 * GL], f32, tag="g", name=f"x{gname}{sg}")
                    for half in range(2):
                        m = gm * 2 + half
                        dst = t[:, half * GL:(half + 1) * GL]
                        for ki in range(3):
                            r = 128 if ki < 2 else IN_DIM - 256
                            nc.tensor.matmul(
                                dst,
                                WxT_sb[0:r, ki * 1024 + m * 128:
                                       ki * 1024 + (m + 1) * 128],
                                embsT_sb[0:r, ki * LPC + sg * GL:
                                         ki * LPC + (sg + 1) * GL],
                                start=(ki == 0), stop=(ki == 2))
                    xg[gname] = t
                ut = gates.tile([128, 2 * GL], f32, tag="u", name=f"u{sg}")
                it = gates.tile([128, 2 * GL], f32, tag="i", name=f"i{sg}")
                ot = gates.tile([128, 2 * GL], f32, tag="o", name=f"o{sg}")
                tht = gates.tile([128, 2 * GL], f32, tag="th", name=f"th{sg}")
                for gname, dst, fn, gm in (("u", ut, AF.Tanh, 0),
                                           ("i", it, AF.Sigmoid, 1),
                                           ("o", ot, AF.Sigmoid, 3)):
                    for half in range(2):
                        nc.scalar.activation(
                            dst[:, half * GL:(half + 1) * GL],
                            xg[gname][:, half * GL:(half + 1) * GL],
                            fn, bias=bx_fm[:, gm * 2 + half:gm * 2 + half + 1])
                cs = c0_3[:, :, sg * GL:(sg + 1) * GL]
                u3, i3, o3, th3 = v2(ut), v2(it), v2(ot), v2(tht)
                nc.vector.tensor_mul(cs, i3, u3)
                nc.scalar.activation(th3, cs, AF.Tanh)
                nc.vector.tensor_mul(hev3[:, :, sg * 256:(sg + 1) * 256],
                                     o3[:, :, 0::2], th3[:, :, 0::2])
                nc.vector.tensor_mul(hod3[:, :, sg * 256:(sg + 1) * 256],
                                     o3[:, :, 1::2], th3[:, :, 1::2])

            # ---- px = pad_row @ Wx.T + bx ----
            px_ps = psum.tile([1, 1024], f32, tag="g")
            for nh in range(2):
                for k in range(3):
                    r = 128 if k < 2 else IN_DIM - 256
                    nc.tensor.matmul(
                        px_ps[:, nh * 512:(nh + 1) * 512],
                        padT_sb[0:r, k:k + 1],
                        WxT_sb[0:r, k * 1024 + nh * 512:k * 1024 + (nh + 1) * 512],
                        start=(k == 0), stop=(k == 2))
            px_sb = const.tile([1, 1024], f32)
            nc.vector.tensor_add(px_sb[:, :], px_ps[:, :], bx_sb[:, :])
            px_fm = const.tile([128, 8], f32)
            for m in range(8):
                tp = psum.tile([128, 1], f32, tag="tp", name=f"pxt{m}")
                nc.tensor.transpose(tp[:, :], px_sb[0:1, m * 128:(m + 1) * 128],
                                    eye_sb[0:1, 0:1])
                nc.scalar.copy(px_fm[:, m:m + 1], tp[:, :])
            px5 = const.tile([1, 1280], f32r)  # node-major 5-gate pad row
            for g in range(5):
                nc.vector.tensor_copy(
                    px5[0:1, 256 * g:256 * (g + 1)],
                    px_sb[0:1, _PX5SRC[g]:_PX5SRC[g] + 256])

            # ---- feature-major level (B >= 256) ----
            def fm_level(cp, hev_p, hod_p, Bp, lvl, split_c):
                B = Bp // 2
                hev_n = state.tile([128, 2 * (B // 2)], f32r, tag="hev",
                                   name=f"hev{lvl}")
                hod_n = state.tile([128, 2 * (B // 2)], f32r, tag="hod",
                                   name=f"hod{lvl}")
                if split_c:
                    cev = state.tile([128, 2 * (B // 2)], f32, tag="cev",
                                     name=f"cev{lvl}", bufs=1)
                    cod = state.tile([128, 2 * (B // 2)], f32, tag="cod",
                                     name=f"cod{lvl}", bufs=1)
                else:
                    cn = state.tile([128, 2 * B], f32, tag="c", name=f"c{lvl}")
                cp3 = v2(cp)
                for g0 in range(0, B, 256):
                    G = min(256, B - g0)
                    gt = []
                    for gi in range(5):
                        t = psum.tile([128, 2 * G], f32, tag="g",
                                      name=f"g{lvl}_{g0}_{gi}")
                        for half in range(2):
                            m = gi * 2 + half
                            dst = t[:, half * G:(half + 1) * G]
                            for ki in range(4):
                                W = WlT_sb if ki < 2 else WrT_sb
                                kc = ki % 2
                                hp = hev_p if ki < 2 else hod_p
                                nc.tensor.matmul(
                                    dst,
                                    W[:, kc * 1280 + m * 128:
                                      kc * 1280 + (m + 1) * 128],
                                    v2(hp)[:, kc, g0:g0 + G],
                                    start=(ki == 0), stop=(ki == 3))
                        gt.append(t)
                    sfx = f"{lvl}_{g0}"
                    ut = gates.tile([128, 2 * G], f32, tag="u", name=f"u{sfx}")
                    it = gates.tile([128, 2 * G], f32, tag="i", name=f"i{sfx}")
                    lft = gates.tile([128, 2 * G], f32, tag="lf", name=f"lf{sfx}")
                    rft = gates.tile([128, 2 * G], f32, tag="rf", name=f"rf{sfx}")
                    ot = gates.tile([128, 2 * G], f32, tag="o", name=f"o{sfx}")
                    tht = gates.tile([128, 2 * G], f32, tag="th", name=f"th{sfx}")
                    x1 = gates.tile([128, 2 * G], f32, tag="x1", name=f"x1{sfx}", bufs=1)
                    x2 = gates.tile([128, 2 * G], f32, tag="x2", name=f"x2{sfx}", bufs=1)
                    x3 = gates.tile([128, 2 * G], f32, tag="x3", name=f"x3{sfx}", bufs=1)
                    s1 = gates.tile([128, 2 * G], f32, tag="s1", name=f"s1{sfx}", bufs=1)
                    for gi, (dst, fn) in enumerate((
                            (ut, AF.Tanh), (it, AF.Sigmoid), (lft, AF.Sigmoid),
                            (rft, AF.Sigmoid), (ot, AF.Sigmoid))):
                        for half in range(2):
                            m = gi * 2 + half
                            nc.scalar.activation(
                                dst[:, half * G:(half + 1) * G],
                                gt[gi][:, half * G:(half + 1) * G],
                                fn, bias=px_fm[:, _PXCOL[m]:_PXCOL[m] + 1])
                    lc = cp3[:, :, 2 * g0:2 * (g0 + G):2]
                    rc = cp3[:, :, 2 * g0 + 1:2 * (g0 + G):2]
                    u3, i3 = v2(ut), v2(it)
                    lf3, rf3, o3, th3 = v2(lft), v2(rft), v2(ot), v2(tht)
                    x13, x23, x33, s13 = v2(x1), v2(x2), v2(x3), v2(s1)
                    nc.vector.tensor_mul(x13, i3, u3)
                    nc.vector.tensor_mul(x23, lf3, lc)
                    nc.vector.tensor_mul(x33, rf3, rc)
                    nc.vector.tensor_add(s13, x13, x23)
                    if split_c:
                        ce = v2(cev)[:, :, g0 // 2:(g0 + G) // 2]
                        co = v2(cod)[:, :, g0 // 2:(g0 + G) // 2]
                        nc.vector.tensor_add(ce, s13[:, :, 0::2], x33[:, :, 0::2])
                        nc.vector.tensor_add(co, s13[:, :, 1::2], x33[:, :, 1::2])
                        nc.scalar.activation(th3[:, :, 0::2], ce, AF.Tanh)
                        nc.scalar.activation(th3[:, :, 1::2], co, AF.Tanh)
                    else:
                        cs = v2(cn)[:, :, g0:g0 + G]
                        nc.vector.tensor_add(cs, s13, x33)
                        nc.scalar.activation(th3, cs, AF.Tanh)
                    nc.vector.tensor_mul(v2(hev_n)[:, :, g0 // 2:(g0 + G) // 2],
                                         o3[:, :, 0::2], th3[:, :, 0::2])
                    nc.vector.tensor_mul(v2(hod_n)[:, :, g0 // 2:(g0 + G) // 2],
                                         o3[:, :, 1::2], th3[:, :, 1::2])
                if split_c:
                    return (cev, cod), hev_n, hod_n, B
                return cn, hev_n, hod_n, B

            # ---- node-major level (B <= 128) ----
            # lcrc: [B, 512] tile, cols [0:256]=lc, [256:512]=rc
            def nm_level(lcrc, hev_p, hod_p, B, lvl, last, ntot=None, hoff=0,
                         tg=""):
                if ntot is None:
                    ntot = B
                g_ps = psum.tile([128, 1280], f32, tag="g", name=f"gn{lvl}{tg}")
                for n0, nw in ((0, 512), (512, 512), (1024, 256)):
                    for ki in range(5):
                        if ki < 4:
                            par, kc = ki // 2, ki % 2
                            hsrc = hev_p if par == 0 else hod_p
                            lhsT = hsrc[:, kc * ntot + hoff:kc * ntot + hoff + B]
                            W = WlT_sb if par == 0 else WrT_sb
                            rhs = W[:, kc * 1280 + n0:kc * 1280 + n0 + nw]
                        else:
                            lhsT = ones_sb[0:1, 0:B]
                            rhs = px5[0:1, n0:n0 + nw]
                        nc.tensor.matmul(g_ps[0:B, n0:n0 + nw], lhsT, rhs,
                                         start=(ki == 0), stop=(ki == 4))
                sfx = f"n{lvl}{tg}"
                ut = gates.tile([128, 256], f32, tag=f"u{tg}", name=f"u{sfx}", bufs=1)
                sig = gates.tile([128, 1024], f32, tag=f"sg{tg}", name=f"sg{sfx}", bufs=1)
                tht = gates.tile([128, 256], f32, tag=f"th{tg}", name=f"th{sfx}", bufs=1)
                x1 = gates.tile([128, 256], f32, tag=f"x1{tg}", name=f"x1{sfx}", bufs=1)
                x23 = gates.tile([128, 512], f32, tag=f"x23{tg}", name=f"x23{sfx}", bufs=1)
                s1 = gates.tile([128, 256], f32, tag=f"s1{tg}", name=f"s1{sfx}", bufs=1)
                c_nm = state.tile([128, 256], f32, tag=f"cn{tg}", name=f"cn{sfx}")
                h_nm = state.tile([128, 256], f32, tag=f"hn{tg}", name=f"hn{sfx}")
                nc.scalar.activation(ut[0:B, :], g_ps[0:B, 0:256], AF.Tanh)
                nc.scalar.activation(sig[0:B, 0:256], g_ps[0:B, 256:512],
                                     AF.Sigmoid)
                nc.scalar.activation(sig[0:B, 256:768], g_ps[0:B, 512:1024],
                                     AF.Sigmoid)
                nc.scalar.activation(sig[0:B, 768:1024], g_ps[0:B, 1024:1280],
                                     AF.Sigmoid)
                nc.vector.tensor_mul(x1[0:B, :], sig[0:B, 0:256], ut[0:B, :])
                nc.vector.tensor_mul(x23[0:B, :], sig[0:B, 256:768], lcrc[0:B, :])
                nc.vector.tensor_add(s1[0:B, :], x1[0:B, :], x23[0:B, 0:256])
                nc.vector.tensor_add(c_nm[0:B, :], s1[0:B, :], x23[0:B, 256:512])
                nc.scalar.activation(tht[0:B, :], c_nm[0:B, :], AF.Tanh)
                nc.vector.tensor_mul(h_nm[0:B, :], sig[0:B, 768:1024], tht[0:B, :])
                if last:
                    return c_nm, h_nm, None, None
                hev_n = state.tile([128, 2 * (B // 2)], f32r, tag=f"hev{tg}",
                                   name=f"hev{lvl}{tg}")
                hod_n = state.tile([128, 2 * (B // 2)], f32r, tag=f"hod{tg}",
                                   name=f"hod{lvl}{tg}")
                for kc in range(2):
                    tp = psum.tile([128, B], f32, tag="tp", name=f"tph{lvl}{tg}_{kc}")
                    nc.tensor.transpose(tp[:, :],
                                        h_nm[0:B, 128 * kc:128 * (kc + 1)],
                                        eye_sb[0:B, 0:B])
                    nc.vector.tensor_copy(
                        hev_n[:, kc * (B // 2):(kc + 1) * (B // 2)],
                        tp[:, 0:B:2])
                    nc.vector.tensor_copy(
                        hod_n[:, kc * (B // 2):(kc + 1) * (B // 2)],
                        tp[:, 1:B:2])
                return c_nm, h_nm, hev_n, hod_n

            def gather_children(c_src, B, lvl, tg=""):
                lcrc = gates.tile([128, 512], f32, tag=f"lcrc{tg}",
                                  name=f"lcrc{lvl}{tg}")
                nc.sync.dma_start(lcrc[0:B, 0:256], c_src[0:2 * B:2, :])
                nc.sync.dma_start(lcrc[0:B, 256:512], c_src[1:2 * B:2, :])
                return lcrc

            # lvl0 (1024->512, FM, contiguous c), lvl1 (512->256, FM, split c)
            c_lvl0, hev, hod, B = fm_level(c0, hev, hod, LPC, 0, False)
            (cev1, cod1), hev, hod, B = fm_level(c_lvl0, hev, hod, B, 1, True)

            # boundary: transpose split FM c into node-major lcrc for lvl2
            lcrc = gates.tile([128, 512], f32, tag="lcrcA", name="lcrc2")
            for par, src in ((0, cev1), (1, cod1)):
                for kc in range(2):
                    tp = psum.tile([128, 128], f32, tag="tp",
                                   name=f"tpb{par}_{kc}")
                    nc.tensor.transpose(tp[:, :], v2(src)[:, kc, :],
                                        eye_sb[:, :])
                    nc.vector.tensor_copy(
                        lcrc[:, 256 * par + 128 * kc:256 * par + 128 * (kc + 1)],
                        tp[:, :])

            # lvl2..lvl9 node-major (B = 128..1)
            hevp, hodp, ntot = hev, hod, 128
            for lvl in range(2, 10):
                B >>= 1  # 128, 64, ..., 1
                last = (lvl == 9)
                c_nm, h_nm, hev_n, hod_n = nm_level(lcrc, hevp, hodp, B, lvl,
                                                    last, ntot=ntot, tg="A")
                if not last:
                    hevp, hodp, ntot = hev_n, hod_n, B // 2
                    lcrc = gather_children(c_nm, B // 2, lvl + 1, "A")

            # ---- write this core's subtree root (c, h) ----
            nc.sync.dma_start(out[0:1, :], c_nm[0:1, :])
            nc.sync.dma_start(out[1:2, :], h_nm[0:1, :])

    nc.compile()
    return nc


def _get_nc():
    if "nc" not in _CACHE:
        _CACHE["nc"] = _build()
    return _CACHE["nc"]


def kernel(embs, Wx, bx, Wl, Wr, emb_table, _trace=False, _trace_kwargs=None):
    from concourse.bass_utils import run_bass_kernel_spmd

    embs = np.ascontiguousarray(np.asarray(embs, dtype=np.float32))
    Wx = np.asarray(Wx, dtype=np.float32)
    bx = np.asarray(bx, dtype=np.float32)
    Wl = np.asarray(Wl, dtype=np.float32)
    Wr = np.asarray(Wr, dtype=np.float32)
    emb_table = np.asarray(emb_table, dtype=np.float32)

    WxT = np.ascontiguousarray(Wx.T)
    WlT = np.ascontiguousarray(Wl.T)
    WrT = np.ascontiguousarray(Wr.T)
    bxr = np.ascontiguousarray(bx.reshape(1, 1024))
    padT = np.ascontiguousarray(emb_table[-1].reshape(IN_DIM, 1))
    eye = np.eye(128, dtype=np.float32)
    ones = np.ones((1, 128), dtype=np.float32)

    in_maps = []
    for d in range(N_CORES):
        shard = np.ascontiguousarray(embs[d * LPC:(d + 1) * LPC].T)
        in_maps.append({
            "embsT": shard, "WxT": WxT, "WlT": WlT, "WrT": WrT,
            "bxr": bxr, "padT": padT, "eye_in": eye, "ones_in": ones,
        })

    nc = _get_nc()
    res = run_bass_kernel_spmd(nc, in_maps, list(range(N_CORES)),
                               trace=_trace, **(_trace_kwargs or {}))
    _CACHE["last_result"] = res

    # unshard: combine the 8 subtree roots (3 merge levels, 7 nodes)
    roots = [np.asarray(res.results[d]["out"], dtype=np.float32)
             for d in range(N_CORES)]
    c = np.stack([r[0] for r in roots])  # [8, 256]
    h = np.stack([r[1] for r in roots])
    px = emb_table[-1] @ WxT + bx        # [1024]
    m = MEM_DIM

    def sig(x):
        return 1.0 / (1.0 + np.exp(-x))

    while c.shape[0] > 1:
        lg = h[0::2] @ WlT
        rg = h[1::2] @ WrT
        u = np.tanh(px[0:m] + lg[:, 0:m] + rg[:, 0:m])
        i = sig(px[m:2 * m] + lg[:, m:2 * m] + rg[:, m:2 * m])
        lf = sig(px[2 * m:3 * m] + lg[:, 2 * m:3 * m] + rg[:, 2 * m:3 * m])
        rf = sig(px[2 * m:3 * m] + lg[:, 3 * m:4 * m] + rg[:, 3 * m:4 * m])
        o = sig(px[3 * m:4 * m] + lg[:, 4 * m:5 * m] + rg[:, 4 * m:5 * m])
        c = i * u + lf * c[0::2] + rf * c[1::2]
        h = o * np.tanh(c)
    return np.stack([c, h]).astype(np.float32)
